# revision 31
# baseline (speedup 1.0000x reference)
"""Trainium2 kernel for nn_Block1SyntaxEngine_85959475462663
(6-layer dense transformer, B=2 T=1024 D=1024 H=16 DFF=2048, fp32 ref).

Distribution: 2-way data-parallel over batch x 4-way SEQUENCE-parallel within
each batch group (core = batch*4 + rank; rank owns tokens
[rank*256,(rank+1)*256)).  Weights are fully replicated and streamed
HBM->SBUF each layer.  The residual lives feature-major ([d, token]) in SBUF:
LayerNorm statistics come from ones-matmuls (free broadcast across
partitions), so no activation transposes are needed anywhere in the layer
loop.  Per layer ONE AllGather moves K (feature-major) + V (token-major);
causal masking is data-driven (per-rank [0/tri/1] mask tiles), keeping the
instruction stream identical on all 8 cores.  Attention is computed k-major:
scores S^T[k_tok, q_tok]; the softmax denominator rides along as an appended
ones-column of V (M=65 AV matmuls); per-token normalization is broadcast
across partitions with K=1 matmuls.

Self-contained: only needs numpy/jax/concourse (the trn_rl_repo toolchain
on sys.path) and 8 visible neuron cores.
"""
import contextlib
import time

import numpy as np

import concourse.bass as bass
import concourse.mybir as mybir
import concourse.tile as tile
from concourse import bacc

P = 128
B, T, D, H, L, V = 2, 1024, 1024, 16, 6, 32000
DH = D // H            # 64
DFF = 2 * D            # 2048
NCORES = 8
NG = 4                 # cores per batch group (sequence-parallel degree)
TOK = T // NG          # 256 own tokens per core
KT = D // P            # 8 contraction tiles over D
FT = DFF // P          # 16
GT = T // P            # 8 global token tiles per batch

f16 = mybir.dt.float16
f32 = mybir.dt.float32
i32 = mybir.dt.int32
AF = mybir.ActivationFunctionType
ALU = mybir.AluOpType
EPS = 1e-5
SIM_GELU_SUBST = False   # True: use Sigmoid instead of Gelu (sim lacks Gelu)
GROUPS = [[0, 1, 2, 3], [4, 5, 6, 7]]


def build_nc():
    nc = bacc.Bacc()
    dp = dict(
        emb=nc.declare_dram_parameter("emb", [V, D], f16, isOutput=False),
        ids=nc.declare_dram_parameter("ids", [TOK, 1], i32, isOutput=False),
        pos=nc.declare_dram_parameter("pos", [TOK, D], f16, isOutput=False),
        wqkv=nc.declare_dram_parameter("wqkv", [L, D, 3 * D], f16, isOutput=False),
        wout=nc.declare_dram_parameter("wout", [L, D, D], f16, isOutput=False),
        w1=nc.declare_dram_parameter("w1", [L, D, DFF], f16, isOutput=False),
        w2=nc.declare_dram_parameter("w2", [L, DFF, D], f16, isOutput=False),
        bqk=nc.declare_dram_parameter("bqk", [L, 16, P], f32, isOutput=False),
        bv=nc.declare_dram_parameter("bv", [L, D], f16, isOutput=False),
        b1=nc.declare_dram_parameter("b1", [L, 16, P], f32, isOutput=False),
        lnf=nc.declare_dram_parameter("lnf", [2, KT, P], f32, isOutput=False),
        masks=nc.declare_dram_parameter("masks", [P, GT, 2, P], f16, isOutput=False),
        ident=nc.declare_dram_parameter("ident", [P, P], f16, isOutput=False),
        out=nc.declare_dram_parameter("out", [TOK, D], f32, isOutput=True),
    )
    with tile.TileContext(nc) as tc:
        _body(nc, tc, dp)
    nc.finalize()
    return nc


def _body(nc, tc, dp):
    ctx = contextlib.ExitStack()
    with ctx:
        cst = ctx.enter_context(tc.tile_pool(name="cst", bufs=1))
        cst2 = ctx.enter_context(tc.tile_pool(name="cst2", bufs=1))
        xp = ctx.enter_context(tc.tile_pool(name="xp", bufs=1))
        hp = ctx.enter_context(tc.tile_pool(name="hp", bufs=1))
        qp = ctx.enter_context(tc.tile_pool(name="qp", bufs=1))
        kp = ctx.enter_context(tc.tile_pool(name="kp", bufs=1))
        vp = ctx.enter_context(tc.tile_pool(name="vp", bufs=1))
        ep = ctx.enter_context(tc.tile_pool(name="ep", bufs=2))
        op = ctx.enter_context(tc.tile_pool(name="op", bufs=1))
        gp = ctx.enter_context(tc.tile_pool(name="gp", bufs=1))
        sp = ctx.enter_context(tc.tile_pool(name="sp", bufs=2))
        lp = ctx.enter_context(tc.tile_pool(name="lp", bufs=1))
        pwqkv = ctx.enter_context(tc.tile_pool(name="pwqkv", bufs=1))
        pwout = ctx.enter_context(tc.tile_pool(name="pwout", bufs=1))
        pw1 = ctx.enter_context(tc.tile_pool(name="pw1", bufs=1))
        pw2 = ctx.enter_context(tc.tile_pool(name="pw2", bufs=1))
        dmp = ctx.enter_context(tc.tile_pool(name="dmp", bufs=2, space="DRAM"))
        ps_a = ctx.enter_context(tc.tile_pool(name="ps_a", bufs=4, space="PSUM"))
        ps_v = ctx.enter_context(tc.tile_pool(name="ps_v", bufs=2, space="PSUM"))
        ps_av = ctx.enter_context(tc.tile_pool(name="ps_av", bufs=2, space="PSUM"))

        # ---- constants ----
        onesD = cst.tile([P, P], f16)          # 1/D everywhere: mean matmuls
        nc.vector.memset(onesD[:], 1.0 / D)
        ident = cst.tile([P, P], f16)
        nc.sync.dma_start(ident[:], dp["ident"][:])
        Mfull = cst.tile([P, GT, 2, P], f16)   # per-rank causal masks
        nc.sync.dma_start(Mfull[:], dp["masks"][:])
        eps_t = cst.tile([P, 1], f32)
        nc.vector.memset(eps_t[:], EPS)
        ones1 = cst.tile([65, DH], f16)
        nc.vector.memset(ones1[:], 1.0)
        bqk_t = cst.tile([P, L, 16], f32)
        nc.sync.dma_start(bqk_t[:], dp["bqk"].rearrange("l m p -> p l m"))
        b1_t = cst.tile([P, L, 16], f32)
        nc.sync.dma_start(b1_t[:], dp["b1"].rearrange("l m p -> p l m"))
        lnf_t = cst.tile([P, 2, KT], f32)
        nc.sync.dma_start(lnf_t[:], dp["lnf"].rearrange("s k p -> p s k"))

        # ---- persistent activations ----
        xT = xp.tile([P, KT, TOK], f32)        # residual, feature-major
        qT = qp.tile([P, KT, TOK], f16)        # q, feature-major
        k_full = kp.tile([P, KT, GT, P], f16)  # K all tokens, feature-major
        v_full = vp.tile([P, GT, H, DH + 1], f16)  # V all tokens, token-major
        nc.vector.memset(v_full[:, :, :, DH], 1.0)  # denominator ones column

        # ---- embeddings: gather + add pos (token-major), transpose to xT ----
        for s in range(2):
            idt = sp.tile([P, 1], i32, tag="ids")
            nc.sync.dma_start(idt[:], dp["ids"][s * P:(s + 1) * P, :])
            gt = ep.tile([P, D], f16, tag="E16")      # reuse E16's 2KB slot
            nc.gpsimd.indirect_dma_start(
                out=gt[:], out_offset=None, in_=dp["emb"][:, :],
                in_offset=bass.IndirectOffsetOnAxis(ap=idt[:, :1], axis=0),
            )
            pt = hp.tile([P, D], f16, tag="h16")      # reuse h16's 4KB slot
            nc.sync.dma_start(pt[:], dp["pos"][s * P:(s + 1) * P, :])
            nc.vector.tensor_add(gt[:], gt[:], pt[:, 0:D])
            for kt in range(KT):
                pst = ps_a.tile([P, 512], f32, tag="ps_a")
                nc.tensor.matmul(pst[:, 0:P], gt[:, kt * P:(kt + 1) * P],
                                 ident[:], start=True, stop=True)
                nc.vector.tensor_copy(xT[:, kt, s * P:(s + 1) * P], pst[:, 0:P])

        def layernorm_T(dst16, final=False):
            """dst16[P,KT,TOK] f16 = LN(xT) feature-major (stats via matmul).
            dst16 doubles as the x16/x^2 staging buffer for the stat matmuls;
            the WAR deps serialize copy -> sum -> square -> sqsum -> h."""
            for k2 in range(KT // 2):
                nc.scalar.activation(dst16[:, 2 * k2:2 * k2 + 2, :],
                                     xT[:, 2 * k2:2 * k2 + 2, :], AF.Copy)
            sps = ps_v.tile([P, 512], f32, tag="ps_v")
            for kt in range(KT):
                nc.tensor.matmul(sps[:, 0:TOK], onesD[:], dst16[:, kt, :],
                                 start=(kt == 0), stop=(kt == KT - 1))
            for k2 in range(KT // 2):
                nc.scalar.activation(dst16[:, 2 * k2:2 * k2 + 2, :],
                                     xT[:, 2 * k2:2 * k2 + 2, :], AF.Square)
            sqs = ps_v.tile([P, 512], f32, tag="ps_v")
            for kt in range(KT):
                nc.tensor.matmul(sqs[:, 0:TOK], onesD[:], dst16[:, kt, :],
                                 start=(kt == 0), stop=(kt == KT - 1))
            var = lp.tile([P, TOK], f32, tag="ln_var")
            nc.scalar.activation(var[:], sps[:, 0:TOK], AF.Square)
            nc.vector.tensor_sub(var[:], sqs[:, 0:TOK], var[:])
            std = lp.tile([P, TOK], f16, tag="ln_std")
            nc.scalar.activation(std[:], var[:], AF.Sqrt, bias=eps_t[:])
            rstd = lp.tile([P, TOK], f16, tag="ln_rstd")
            with nc.allow_low_precision(reason="LN rstd fp16 ok at 2e-2 tol"):
                nc.vector.reciprocal(rstd[:], std[:])
            for kt in range(KT):
                nc.vector.tensor_sub(dst16[:, kt, :], xT[:, kt, :], sps[:, 0:TOK])
                nc.vector.tensor_mul(dst16[:, kt, :], dst16[:, kt, :], rstd[:])
                if final:
                    nc.vector.tensor_scalar(
                        dst16[:, kt, :], dst16[:, kt, :],
                        lnf_t[:, 0, kt:kt + 1], lnf_t[:, 1, kt:kt + 1],
                        ALU.mult, ALU.add)

        # ================= layers =================
        for l in range(L):
            wqkv_t = pwqkv.tile([P, KT, 3 * D], f16, tag="wqkv")
            nc.sync.dma_start(
                wqkv_t[:], dp["wqkv"][l].rearrange("(kt p) f -> p kt f", p=P))
            wout_t = pwout.tile([P, KT, D], f16, tag="wout")
            nc.sync.dma_start(
                wout_t[:], dp["wout"][l].rearrange("(kt p) f -> p kt f", p=P))
            w1_t = pw1.tile([P, KT, DFF], f16, tag="w1")
            nc.sync.dma_start(
                w1_t[:], dp["w1"][l].rearrange("(kt p) f -> p kt f", p=P))
            w2_t = pw2.tile([P, FT, D], f16, tag="w2")
            nc.sync.dma_start(
                w2_t[:], dp["w2"][l].rearrange("(ft p) f -> p ft f", p=P))
            bv_bc = cst2.tile([P, H, DH], f16, tag="bv")
            nc.sync.dma_start(
                bv_bc[:].rearrange("p h d -> p (h d)"),
                dp["bv"][l, None, :].to_broadcast((P, D)))

            h16 = hp.tile([P, KT, TOK], f16, tag="h16")
            layernorm_T(h16)

            # ---- K/V projections + AllGather, split into head halves so the
            # second AG hides behind attention on heads 0-7 ----
            kv_io = []
            for hh in range(2):
                for ft in range(4 * hh, 4 * hh + 4):
                    fb = 8 + ft
                    ps = ps_a.tile([P, 512], f32, tag="ps_a")
                    for kt in range(KT):
                        nc.tensor.matmul(
                            ps[:, 0:TOK], wqkv_t[:, kt, fb * P:(fb + 1) * P],
                            h16[:, kt, :], start=(kt == 0), stop=(kt == KT - 1))
                    for s in range(2):
                        nc.vector.tensor_scalar_add(
                            k_full[:, ft, s, :], ps[:, s * P:(s + 1) * P],
                            bqk_t[:, l, fb:fb + 1])
                fc = hh
                for s in range(2):
                    psv = ps_v.tile([P, 512], f32, tag="ps_v")
                    for kt in range(KT):
                        nc.tensor.matmul(
                            psv[:], h16[:, kt, s * P:(s + 1) * P],
                            wqkv_t[:, kt, 2 * D + fc * 512:2 * D + (fc + 1) * 512],
                            start=(kt == 0), stop=(kt == KT - 1))
                    nc.vector.tensor_add(
                        v_full[:, s, fc * 8:(fc + 1) * 8, 0:DH],
                        psv[:].rearrange("p (h d) -> p h d", h=8),
                        bv_bc[:, fc * 8:(fc + 1) * 8, :])
                kv_i = dmp.tile([P, 2048], f16, tag=f"kvi{hh}")
                kv_o = dmp.tile([NG, P, 2048], f16, tag=f"kvo{hh}")
                nc.sync.dma_start(
                    kv_i[:, 0:1024].rearrange("p (ft s t) -> p ft s t", ft=4, s=2),
                    k_full[:, 4 * hh:4 * hh + 4, 0:2, :])
                for s in range(2):
                    nc.sync.dma_start(
                        kv_i[:, 1024 + s * 512:1536 + s * 512].rearrange(
                            "p (h d) -> p h d", h=8),
                        v_full[:, s, 8 * hh:8 * hh + 8, 0:DH])
                nc.gpsimd.collective_compute(
                    "AllGather", ALU.bypass, replica_groups=GROUPS,
                    ins=[kv_i[:]], outs=[kv_o[:]],
                )
                kv_io.append(kv_o)

            # ---- Q projection (overlaps the AllGathers) ----
            for fb in range(KT):
                ps = ps_a.tile([P, 512], f32, tag="ps_a")
                for kt in range(KT):
                    nc.tensor.matmul(
                        ps[:, 0:TOK], wqkv_t[:, kt, fb * P:(fb + 1) * P],
                        h16[:, kt, :], start=(kt == 0), stop=(kt == KT - 1))
                nc.vector.tensor_scalar_add(qT[:, fb, :], ps[:, 0:TOK],
                                            bqk_t[:, l, fb:fb + 1])

            # ---- unpack gathered K,V (per half) ----
            for hh in range(2):
                kv_o = kv_io[hh]
                for rho in range(NG):
                    nc.sync.dma_start(
                        k_full[:, 4 * hh:4 * hh + 4, 2 * rho:2 * rho + 2, :],
                        kv_o[rho, :, 0:1024].rearrange(
                            "p (ft s t) -> p ft s t", ft=4, s=2))
                    for s in range(2):
                        nc.sync.dma_start(
                            v_full[:, 2 * rho + s, 8 * hh:8 * hh + 8, 0:DH],
                            kv_o[rho, :, 1024 + s * 512:1536 + s * 512].rearrange(
                                "p (h d) -> p h d", h=8))

            # ---- attention, k-major ----
            oT = op.tile([P, KT, TOK], f16, tag="oT")        # paired heads
            for h in range(H):
                ft, r0 = h // 2, (h % 2) * DH
                if h % 8 == 0:
                    oTmp = op.tile([DH, 4, TOK], f16, tag="oTmp")
                psav = ps_av.tile([P, 512], f32, tag="ps_av")
                for w in range(2):           # two j-waves of 4 tiles each
                    E16 = ep.tile([P, 4, TOK], f16, tag="E16")
                    for jp in range(2):
                        j0 = 4 * w + 2 * jp
                        ps = ps_a.tile([P, 512], f32, tag="ps_a")
                        for dj in range(2):
                            nc.tensor.matmul(
                                ps[:, dj * TOK:(dj + 1) * TOK],
                                k_full[r0:r0 + DH, ft, j0 + dj, :],
                                qT[r0:r0 + DH, ft, :], start=True, stop=True)
                        nc.scalar.activation(
                            E16[:, 2 * jp:2 * jp + 2, :],
                            ps[:].rearrange("p (a b) -> p a b", a=2),
                            AF.Exp, scale=float(1.0 / np.sqrt(DH)))
                        nc.gpsimd.tensor_mul(
                            E16[:, 2 * jp:2 * jp + 2, :].rearrange(
                                "p a (s t) -> p a s t", s=2),
                            E16[:, 2 * jp:2 * jp + 2, :].rearrange(
                                "p a (s t) -> p a s t", s=2),
                            Mfull[:, j0:j0 + 2, :, :])
                    for dj in range(4):
                        j = 4 * w + dj
                        nc.tensor.matmul(
                            psav[0:DH + 1, 0:TOK], v_full[:, j, h, :],
                            E16[:, dj, :], start=(j == 0), stop=(j == GT - 1))
                rn = sp.tile([65, TOK], f16, tag="rn")
                with nc.allow_low_precision(reason="softmax denom fp16 ok at 2e-2 tol"):
                    nc.vector.reciprocal(rn[64:65, :], psav[64:65, 0:TOK])
                o_un = sp.tile([DH, TOK], f16, tag="rnb")
                nc.scalar.activation(o_un[:], psav[0:DH, 0:TOK], AF.Copy)
                bc = ps_v.tile([P, 512], f32, tag="ps_v")
                nc.tensor.matmul(bc[0:DH, 0:TOK], ones1[64:65, 0:DH],
                                 rn[64:65, :], start=True, stop=True)
                dst = oT[0:DH, ft, :] if h % 2 == 0 else oTmp[:, ft % 4, :]
                nc.vector.tensor_mul(dst, o_un[:], bc[0:DH, 0:TOK])
                if h % 8 == 7:
                    nc.sync.dma_start(
                        oT[DH:P, 4 * (h // 8):4 * (h // 8) + 4, :], oTmp[:])

            # ---- Wout (K=128, paired heads) + residual add ----
            for db in range(KT):
                ps = ps_a.tile([P, 512], f32, tag="ps_a")
                for ot in range(KT):
                    nc.tensor.matmul(
                        ps[:, 0:TOK], wout_t[:, ot, db * P:(db + 1) * P],
                        oT[:, ot, :], start=(ot == 0), stop=(ot == KT - 1))
                nc.vector.tensor_add(xT[:, db, :], xT[:, db, :], ps[:, 0:TOK])

            # ---- FFN ----
            h2 = hp.tile([P, KT, TOK], f16, tag="h16")
            layernorm_T(h2)
            for ffh in range(2):
                g16 = gp.tile([P, KT, TOK], f16, tag="g16")
                for fo in range(KT):
                    ffb = ffh * KT + fo
                    ps = ps_a.tile([P, 512], f32, tag="ps_a")
                    for kt in range(KT):
                        nc.tensor.matmul(
                            ps[:, 0:TOK], w1_t[:, kt, ffb * P:(ffb + 1) * P],
                            h2[:, kt, :], start=(kt == 0), stop=(kt == KT - 1))
                    nc.scalar.activation(
                        g16[:, fo, :], ps[:, 0:TOK],
                        AF.Sigmoid if SIM_GELU_SUBST else AF.Gelu,
                        bias=b1_t[:, l, ffb:ffb + 1])
                for db in range(KT):
                    ps = ps_a.tile([P, 512], f32, tag="ps_a")
                    for fo in range(KT):
                        nc.tensor.matmul(
                            ps[:, 0:TOK],
                            w2_t[:, ffh * KT + fo, db * P:(db + 1) * P],
                            g16[:, fo, :], start=(fo == 0), stop=(fo == KT - 1))
                    nc.vector.tensor_add(xT[:, db, :], xT[:, db, :], ps[:, 0:TOK])

        # ---- final layernorm (feature-major) + transpose + output ----
        hf = hp.tile([P, KT, TOK], f16, tag="h16")
        layernorm_T(hf, final=True)
        for s in range(2):
            o32 = gp.tile([P, KT, P], f32, tag="g16")
            for kt in range(KT):
                ps = ps_a.tile([P, 512], f32, tag="ps_a")
                nc.tensor.matmul(ps[:, 0:P], hf[:, kt, s * P:(s + 1) * P],
                                 ident[:], start=True, stop=True)
                nc.vector.tensor_copy(o32[:, kt, :], ps[:, 0:P])
            nc.sync.dma_start(
                dp["out"][s * P:(s + 1) * P, :].rearrange(
                    "t (kt p) -> t kt p", p=P), o32[:])


# ======================= host side =======================

def _prep_inputs(input_ids, token_emb, pos_emb, ln1_s, ln1_b, Wqkv, Wout,
                 ln2_s, ln2_b, W1, W2, lnf_s, lnf_b):
    emb16 = np.asarray(token_emb, np.float16)
    pos32 = np.asarray(pos_emb, np.float32)
    ids_np = np.asarray(input_ids).astype(np.int32)
    Wqkv64 = np.asarray(Wqkv, np.float64)
    W164 = np.asarray(W1, np.float64)
    Wqkv_f = Wqkv64 * np.asarray(ln1_s, np.float64)[:, :, None]
    bqkv_f = np.einsum("ld,ldn->ln", np.asarray(ln1_b, np.float64), Wqkv64)
    W1_f = W164 * np.asarray(ln2_s, np.float64)[:, :, None]
    b1_f = np.einsum("ld,ldn->ln", np.asarray(ln2_b, np.float64), W164)

    wqkv16 = Wqkv_f.astype(np.float16)
    wout16 = np.asarray(Wout, np.float16)
    w116 = W1_f.astype(np.float16)
    w216 = np.asarray(W2, np.float16)
    bqk = bqkv_f[:, 0:2 * D].reshape(L, 16, P).astype(np.float32)
    bv = bqkv_f[:, 2 * D:].astype(np.float16)
    b1 = b1_f.reshape(L, 16, P).astype(np.float32)
    lnf = np.stack([np.asarray(lnf_s, np.float32),
                    np.asarray(lnf_b, np.float32)]).reshape(2, KT, P)
    ident_np = np.eye(P, dtype=np.float16)
    tri = np.triu(np.ones((P, P), np.float32))  # [k, q]: 1 where k <= q

    in_maps = []
    for core in range(NCORES):
        g, r = divmod(core, NG)
        tok0 = r * TOK
        masks = np.zeros((P, GT, 2, P), np.float32)
        for j in range(GT):
            for s in range(2):
                gq = 2 * r + s
                if j < gq:
                    masks[:, j, s, :] = 1.0
                elif j == gq:
                    masks[:, j, s, :] = tri
        in_maps.append(dict(
            emb=emb16,
            ids=ids_np[g, tok0:tok0 + TOK][:, None],
            pos=pos32[tok0:tok0 + TOK].astype(np.float16),
            wqkv=wqkv16, wout=wout16, w1=w116, w2=w216,
            bqk=bqk, bv=bv, b1=b1, lnf=lnf,
            masks=masks.astype(np.float16), ident=ident_np,
        ))
    return in_maps


# ---------- compile-once / run-many PJRT runner (vendored) ----------

class SpmdRunner:
    def __init__(self, nc, n_cores=8):
        import jax
        from jax.sharding import Mesh, PartitionSpec
        from jax.experimental.shard_map import shard_map
        from concourse.bass2jax import (
            _bass_exec_p, install_neuronx_cc_hook, partition_id_tensor)
        self.jax = jax
        self.PartitionSpec = PartitionSpec
        install_neuronx_cc_hook()
        if not nc.is_finalized():
            nc.finalize()
        self.n_cores = n_cores
        partition_name = (
            nc.partition_id_tensor.name if nc.partition_id_tensor else None)
        in_names, out_names, out_avals, zero_outs = [], [], [], []
        for alloc in nc.m.functions[0].allocations:
            if not isinstance(alloc, mybir.MemoryLocationSet):
                continue
            name = alloc.memorylocations[0].name
            if alloc.kind == "ExternalInput":
                if name != partition_name:
                    in_names.append(name)
            elif alloc.kind == "ExternalOutput":
                out_names.append(name)
                shape = tuple(alloc.tensor_shape)
                dtype = mybir.dt.np(alloc.dtype)
                out_avals.append(jax.core.ShapedArray(shape, dtype))
                zero_outs.append(np.zeros(shape, dtype))
        self.in_names, self.out_names = in_names, out_names
        self.out_avals, self.zero_outs = out_avals, zero_outs
        n_params, n_outs = len(in_names), len(out_avals)
        self.n_params = n_params
        all_in = in_names + out_names + (
            [partition_name] if partition_name else [])
        donate = tuple(range(n_params, n_params + n_outs))

        def _b(*args):
            ops = list(args)
            if partition_name:
                ops.append(partition_id_tensor())
            return tuple(_bass_exec_p.bind(
                *ops, out_avals=tuple(out_avals), in_names=tuple(all_in),
                out_names=tuple(out_names), lowering_input_output_aliases=(),
                sim_require_finite=True, sim_require_nnan=True, nc=nc))

        devices = jax.devices()[:n_cores]
        self.mesh = Mesh(np.asarray(devices), ("core",))
        specs = (PartitionSpec("core"),)
        self.sharded = jax.jit(
            shard_map(_b, mesh=self.mesh,
                      in_specs=specs * (n_params + n_outs),
                      out_specs=specs * len(out_names), check_rep=False),
            donate_argnums=donate, keep_unused=True)
        self._dev_inputs = None

    def _zeros(self):
        return [np.zeros((self.n_cores * z.shape[0], *z.shape[1:]), z.dtype)
                for z in self.zero_outs]

    def stage_inputs(self, in_maps):
        jax, PS = self.jax, self.PartitionSpec
        per_core = [[np.asarray(m[n]) for n in self.in_names] for m in in_maps]
        concat = [np.concatenate([per_core[c][i] for c in range(self.n_cores)],
                                 axis=0) for i in range(self.n_params)]
        sh = jax.sharding.NamedSharding(self.mesh, PS("core"))
        self._dev_inputs = [jax.device_put(a, sh) for a in concat]
        for a in self._dev_inputs:
            a.block_until_ready()

    def run(self, in_maps=None):
        if in_maps is not None:
            self.stage_inputs(in_maps)
        outs = self.sharded(*self._dev_inputs, *self._zeros())
        out_np = [np.asarray(a) for a in outs]
        return [{n: out_np[i].reshape(self.n_cores, *self.out_avals[i].shape)[c]
                 for i, n in enumerate(self.out_names)}
                for c in range(self.n_cores)]

    def time_exec(self, iters=8, warmup=2):
        jax, PS = self.jax, self.PartitionSpec
        sh = jax.sharding.NamedSharding(self.mesh, PS("core"))
        zsets = [[jax.device_put(z, sh) for z in self._zeros()]
                 for _ in range(warmup + iters)]
        for zs in zsets:
            for z in zs:
                z.block_until_ready()
        outs = []
        for i in range(warmup):
            outs.append(self.sharded(*self._dev_inputs, *zsets[i]))
        for o in outs[-1]:
            o.block_until_ready()
        t0 = time.perf_counter()
        outs = []
        for i in range(iters):
            outs.append(self.sharded(*self._dev_inputs, *zsets[warmup + i]))
        for o in outs[-1]:
            o.block_until_ready()
        return (time.perf_counter() - t0) / iters


_RUNNER = None


def get_runner():
    global _RUNNER
    if _RUNNER is None:
        _RUNNER = SpmdRunner(build_nc(), NCORES)
    return _RUNNER


def kernel(**inputs) -> np.ndarray:
    in_maps = _prep_inputs(**{k: np.asarray(v) for k, v in inputs.items()})
    res = get_runner().run(in_maps)
    out = np.empty((B, T, D), np.float32)
    for core in range(NCORES):
        g, r = divmod(core, NG)
        out[g, r * TOK:(r + 1) * TOK] = res[core]["out"]
    return out


# revision 33
# speedup vs baseline: 1.7631x; 1.7631x over previous
"""Trainium2 kernel for nn_Block1SyntaxEngine_85959475462663
(6-layer dense transformer, B=2 T=1024 D=1024 H=16 DFF=2048, fp32 ref).

Distribution: 2-way data-parallel over batch x 4-way SEQUENCE-parallel within
each batch group (core = batch*4 + rank; rank owns tokens
[rank*256,(rank+1)*256)).  Weights are fully replicated and streamed
HBM->SBUF each layer.  The residual lives feature-major ([d, token]) in SBUF:
LayerNorm statistics come from ones-matmuls (free broadcast across
partitions), so no activation transposes are needed anywhere in the layer
loop.  Per layer ONE AllGather moves K (feature-major) + V (token-major);
causal masking is data-driven (per-rank [0/tri/1] mask tiles), keeping the
instruction stream identical on all 8 cores.  Attention is computed k-major:
scores S^T[k_tok, q_tok]; the softmax denominator rides along as an appended
ones-column of V (M=65 AV matmuls); per-token normalization is broadcast
across partitions with K=1 matmuls.

Self-contained: only needs numpy/jax/concourse (the trn_rl_repo toolchain
on sys.path) and 8 visible neuron cores.
"""
import contextlib
import time

import numpy as np

import concourse.bass as bass
import concourse.mybir as mybir
import concourse.tile as tile
from concourse import bacc

P = 128
B, T, D, H, L, V = 2, 1024, 1024, 16, 6, 32000
DH = D // H            # 64
DFF = 2 * D            # 2048
NCORES = 8
NG = 4                 # cores per batch group (sequence-parallel degree)
TOK = T // NG          # 256 own tokens per core
KT = D // P            # 8 contraction tiles over D
FT = DFF // P          # 16
GT = T // P            # 8 global token tiles per batch

f16 = mybir.dt.float16
f32 = mybir.dt.float32
i32 = mybir.dt.int32
AF = mybir.ActivationFunctionType
ALU = mybir.AluOpType
EPS = 1e-5
SIM_GELU_SUBST = False   # True: use Sigmoid instead of Gelu (sim lacks Gelu)
GROUPS = [[0, 1, 2, 3], [4, 5, 6, 7]]


def build_nc():
    nc = bacc.Bacc()
    dp = dict(
        emb=nc.declare_dram_parameter("emb", [V, D], f16, isOutput=False),
        ids=nc.declare_dram_parameter("ids", [TOK, 1], i32, isOutput=False),
        pos=nc.declare_dram_parameter("pos", [TOK, D], f16, isOutput=False),
        wqkv=nc.declare_dram_parameter("wqkv", [L, D, 3 * D], f16, isOutput=False),
        wout=nc.declare_dram_parameter("wout", [L, D, D], f16, isOutput=False),
        w1=nc.declare_dram_parameter("w1", [L, D, DFF], f16, isOutput=False),
        w2=nc.declare_dram_parameter("w2", [L, DFF, D], f16, isOutput=False),
        bqk=nc.declare_dram_parameter("bqk", [L, 16, P], f32, isOutput=False),
        bv=nc.declare_dram_parameter("bv", [L, D], f16, isOutput=False),
        b1=nc.declare_dram_parameter("b1", [L, 16, P], f32, isOutput=False),
        lnf=nc.declare_dram_parameter("lnf", [2, KT, P], f32, isOutput=False),
        masks=nc.declare_dram_parameter("masks", [P, GT, 2, P], f16, isOutput=False),
        ident=nc.declare_dram_parameter("ident", [P, P], f16, isOutput=False),
        out=nc.declare_dram_parameter("out", [TOK, D], f32, isOutput=True),
    )
    with tile.TileContext(nc) as tc:
        _body(nc, tc, dp)
    nc.finalize()
    return nc


def _body(nc, tc, dp):
    ctx = contextlib.ExitStack()
    with ctx:
        cst = ctx.enter_context(tc.tile_pool(name="cst", bufs=1))
        cst2 = ctx.enter_context(tc.tile_pool(name="cst2", bufs=1))
        xp = ctx.enter_context(tc.tile_pool(name="xp", bufs=1))
        hp = ctx.enter_context(tc.tile_pool(name="hp", bufs=1))
        qp = ctx.enter_context(tc.tile_pool(name="qp", bufs=1))
        kp = ctx.enter_context(tc.tile_pool(name="kp", bufs=1))
        vp = ctx.enter_context(tc.tile_pool(name="vp", bufs=1))
        ep = ctx.enter_context(tc.tile_pool(name="ep", bufs=2))
        op = ctx.enter_context(tc.tile_pool(name="op", bufs=1))
        gp = ctx.enter_context(tc.tile_pool(name="gp", bufs=1))
        sp = ctx.enter_context(tc.tile_pool(name="sp", bufs=4))
        lp = ctx.enter_context(tc.tile_pool(name="lp", bufs=1))
        pwqkv = ctx.enter_context(tc.tile_pool(name="pwqkv", bufs=1))
        pwout = ctx.enter_context(tc.tile_pool(name="pwout", bufs=1))
        pw1 = ctx.enter_context(tc.tile_pool(name="pw1", bufs=1))
        pw2 = ctx.enter_context(tc.tile_pool(name="pw2", bufs=1))
        dmp = ctx.enter_context(tc.tile_pool(name="dmp", bufs=2, space="DRAM"))
        ps_a = ctx.enter_context(tc.tile_pool(name="ps_a", bufs=4, space="PSUM"))
        ps_v = ctx.enter_context(tc.tile_pool(name="ps_v", bufs=2, space="PSUM"))
        ps_av = ctx.enter_context(tc.tile_pool(name="ps_av", bufs=2, space="PSUM"))

        # ---- constants ----
        onesD = cst.tile([P, P], f16)          # 1/D everywhere: mean matmuls
        nc.vector.memset(onesD[:], 1.0 / D)
        ident = cst.tile([P, P], f16)
        nc.sync.dma_start(ident[:], dp["ident"][:])
        Mfull = cst.tile([P, GT, 2, P], f16)   # per-rank causal masks
        nc.sync.dma_start(Mfull[:], dp["masks"][:])
        eps_t = cst.tile([P, 1], f32)
        nc.vector.memset(eps_t[:], EPS)
        bqk_t = cst.tile([P, L, 16], f32)
        nc.sync.dma_start(bqk_t[:], dp["bqk"].rearrange("l m p -> p l m"))
        b1_t = cst.tile([P, L, 16], f32)
        nc.sync.dma_start(b1_t[:], dp["b1"].rearrange("l m p -> p l m"))
        lnf_t = cst.tile([P, 2, KT], f32)
        nc.sync.dma_start(lnf_t[:], dp["lnf"].rearrange("s k p -> p s k"))

        # ---- persistent activations ----
        xT = xp.tile([P, KT, TOK], f32)        # residual, feature-major
        qT = qp.tile([P, KT, TOK], f16)        # q, feature-major
        k_full = kp.tile([P, KT, GT, P], f16)  # K all tokens, feature-major
        v_full = vp.tile([P, GT, H, DH + 1], f16)  # V all tokens, token-major
        nc.vector.memset(v_full[:, :, :, DH], 1.0)  # denominator ones column

        # ---- embeddings: gather + add pos (token-major), transpose to xT ----
        for s in range(2):
            idt = sp.tile([P, 1], i32, tag="ids")
            nc.sync.dma_start(idt[:], dp["ids"][s * P:(s + 1) * P, :])
            gt = ep.tile([P, D], f16, tag="E16")      # reuse E16's 2KB slot
            nc.gpsimd.indirect_dma_start(
                out=gt[:], out_offset=None, in_=dp["emb"][:, :],
                in_offset=bass.IndirectOffsetOnAxis(ap=idt[:, :1], axis=0),
            )
            pt = hp.tile([P, D], f16, tag="h16")      # reuse h16's 4KB slot
            nc.sync.dma_start(pt[:], dp["pos"][s * P:(s + 1) * P, :])
            nc.vector.tensor_add(gt[:], gt[:], pt[:, 0:D])
            for kt in range(KT):
                pst = ps_a.tile([P, 512], f32, tag="ps_a")
                nc.tensor.matmul(pst[:, 0:P], gt[:, kt * P:(kt + 1) * P],
                                 ident[:], start=True, stop=True)
                nc.vector.tensor_copy(xT[:, kt, s * P:(s + 1) * P], pst[:, 0:P])

        def layernorm_T(dst16, final=False):
            """dst16[P,KT,TOK] f16 = LN(xT) feature-major (stats via matmul).
            dst16 doubles as the x16/x^2 staging buffer for the stat matmuls;
            the WAR deps serialize copy -> sum -> square -> sqsum -> h."""
            for k2 in range(KT // 2):
                nc.scalar.activation(dst16[:, 2 * k2:2 * k2 + 2, :],
                                     xT[:, 2 * k2:2 * k2 + 2, :], AF.Copy)
            sps = ps_v.tile([P, 512], f32, tag="ps_v")
            for kt in range(KT):
                nc.tensor.matmul(sps[:, 0:TOK], onesD[:], dst16[:, kt, :],
                                 start=(kt == 0), stop=(kt == KT - 1))
            for k2 in range(KT // 2):
                nc.scalar.activation(dst16[:, 2 * k2:2 * k2 + 2, :],
                                     xT[:, 2 * k2:2 * k2 + 2, :], AF.Square)
            sqs = ps_v.tile([P, 512], f32, tag="ps_v")
            for kt in range(KT):
                nc.tensor.matmul(sqs[:, 0:TOK], onesD[:], dst16[:, kt, :],
                                 start=(kt == 0), stop=(kt == KT - 1))
            var = lp.tile([P, TOK], f32, tag="ln_var")
            nc.scalar.activation(var[:], sps[:, 0:TOK], AF.Square)
            nc.vector.tensor_sub(var[:], sqs[:, 0:TOK], var[:])
            std = lp.tile([P, TOK], f32, tag="ln_std")
            nc.scalar.activation(std[:], var[:], AF.Sqrt, bias=eps_t[:])
            rstd = lp.tile([P, TOK], f32, tag="ln_rstd")
            nc.vector.reciprocal(rstd[:], std[:])
            for kt in range(KT):
                nc.vector.tensor_sub(dst16[:, kt, :], xT[:, kt, :], sps[:, 0:TOK])
                nc.vector.tensor_mul(dst16[:, kt, :], dst16[:, kt, :], rstd[:])
                if final:
                    nc.vector.tensor_scalar(
                        dst16[:, kt, :], dst16[:, kt, :],
                        lnf_t[:, 0, kt:kt + 1], lnf_t[:, 1, kt:kt + 1],
                        ALU.mult, ALU.add)

        # ================= layers =================
        for l in range(L):
            wqkv_t = pwqkv.tile([P, KT, 3 * D], f16, tag="wqkv")
            nc.sync.dma_start(
                wqkv_t[:], dp["wqkv"][l].rearrange("(kt p) f -> p kt f", p=P))
            wout_t = pwout.tile([P, KT, D], f16, tag="wout")
            nc.sync.dma_start(
                wout_t[:], dp["wout"][l].rearrange("(kt p) f -> p kt f", p=P))
            w1_t = pw1.tile([P, KT, DFF], f16, tag="w1")
            nc.sync.dma_start(
                w1_t[:], dp["w1"][l].rearrange("(kt p) f -> p kt f", p=P))
            w2_t = pw2.tile([P, FT, D], f16, tag="w2")
            nc.sync.dma_start(
                w2_t[:], dp["w2"][l].rearrange("(ft p) f -> p ft f", p=P))
            bv_bc = cst2.tile([P, H, DH], f16, tag="bv")
            nc.sync.dma_start(
                bv_bc[:].rearrange("p h d -> p (h d)"),
                dp["bv"][l, None, :].to_broadcast((P, D)))

            h16 = hp.tile([P, KT, TOK], f16, tag="h16")
            layernorm_T(h16)

            # ---- K/V projections + AllGather, split into head halves so the
            # second AG hides behind attention on heads 0-7 ----
            kv_io = []
            for hh in range(2):
                for ft in range(4 * hh, 4 * hh + 4):
                    fb = 8 + ft
                    ps = ps_a.tile([P, 512], f32, tag="ps_a")
                    for kt in range(KT):
                        nc.tensor.matmul(
                            ps[:, 0:TOK], wqkv_t[:, kt, fb * P:(fb + 1) * P],
                            h16[:, kt, :], start=(kt == 0), stop=(kt == KT - 1))
                    for s in range(2):
                        nc.vector.tensor_scalar_add(
                            k_full[:, ft, s, :], ps[:, s * P:(s + 1) * P],
                            bqk_t[:, l, fb:fb + 1])
                fc = hh
                for s in range(2):
                    psv = ps_v.tile([P, 512], f32, tag="ps_v")
                    for kt in range(KT):
                        nc.tensor.matmul(
                            psv[:], h16[:, kt, s * P:(s + 1) * P],
                            wqkv_t[:, kt, 2 * D + fc * 512:2 * D + (fc + 1) * 512],
                            start=(kt == 0), stop=(kt == KT - 1))
                    nc.vector.tensor_add(
                        v_full[:, s, fc * 8:(fc + 1) * 8, 0:DH],
                        psv[:].rearrange("p (h d) -> p h d", h=8),
                        bv_bc[:, fc * 8:(fc + 1) * 8, :])
                kv_i = dmp.tile([P, 2048], f16, tag=f"kvi{hh}")
                kv_o = dmp.tile([NG, P, 2048], f16, tag=f"kvo{hh}")
                nc.sync.dma_start(
                    kv_i[:, 0:1024].rearrange("p (ft s t) -> p ft s t", ft=4, s=2),
                    k_full[:, 4 * hh:4 * hh + 4, 0:2, :])
                for s in range(2):
                    nc.sync.dma_start(
                        kv_i[:, 1024 + s * 512:1536 + s * 512].rearrange(
                            "p (h d) -> p h d", h=8),
                        v_full[:, s, 8 * hh:8 * hh + 8, 0:DH])
                nc.gpsimd.collective_compute(
                    "AllGather", ALU.bypass, replica_groups=GROUPS,
                    ins=[kv_i[:]], outs=[kv_o[:]],
                )
                kv_io.append(kv_o)

            # ---- Q projection (overlaps the AllGathers) ----
            for fb in range(KT):
                ps = ps_a.tile([P, 512], f32, tag="ps_a")
                for kt in range(KT):
                    nc.tensor.matmul(
                        ps[:, 0:TOK], wqkv_t[:, kt, fb * P:(fb + 1) * P],
                        h16[:, kt, :], start=(kt == 0), stop=(kt == KT - 1))
                nc.vector.tensor_scalar_add(qT[:, fb, :], ps[:, 0:TOK],
                                            bqk_t[:, l, fb:fb + 1])

            # ---- unpack gathered K,V (per half) ----
            for hh in range(2):
                kv_o = kv_io[hh]
                for rho in range(NG):
                    nc.sync.dma_start(
                        k_full[:, 4 * hh:4 * hh + 4, 2 * rho:2 * rho + 2, :],
                        kv_o[rho, :, 0:1024].rearrange(
                            "p (ft s t) -> p ft s t", ft=4, s=2))
                    for s in range(2):
                        nc.sync.dma_start(
                            v_full[:, 2 * rho + s, 8 * hh:8 * hh + 8, 0:DH],
                            kv_o[rho, :, 1024 + s * 512:1536 + s * 512].rearrange(
                                "p (h d) -> p h d", h=8))

            # ---- attention, k-major ----
            oT = op.tile([P, KT, TOK], f16, tag="oT")        # paired heads
            for h in range(H):
                ft, r0 = h // 2, (h % 2) * DH
                if h % 8 == 0:
                    oTmp = op.tile([DH, 4, TOK], f16, tag="oTmp")
                psav = ps_av.tile([P, 512], f32, tag="ps_av")
                for w in range(2):           # two j-waves of 4 tiles each
                    E16 = ep.tile([P, 4, TOK], f16, tag="E16")
                    for jp in range(2):
                        j0 = 4 * w + 2 * jp
                        ps = ps_a.tile([P, 512], f32, tag="ps_a")
                        for dj in range(2):
                            nc.tensor.matmul(
                                ps[:, dj * TOK:(dj + 1) * TOK],
                                k_full[r0:r0 + DH, ft, j0 + dj, :],
                                qT[r0:r0 + DH, ft, :], start=True, stop=True)
                        nc.scalar.activation(
                            E16[:, 2 * jp:2 * jp + 2, :],
                            ps[:].rearrange("p (a b) -> p a b", a=2),
                            AF.Exp, scale=float(1.0 / np.sqrt(DH)))
                        nc.gpsimd.tensor_mul(
                            E16[:, 2 * jp:2 * jp + 2, :].rearrange(
                                "p a (s t) -> p a s t", s=2),
                            E16[:, 2 * jp:2 * jp + 2, :].rearrange(
                                "p a (s t) -> p a s t", s=2),
                            Mfull[:, j0:j0 + 2, :, :])
                    for dj in range(4):
                        j = 4 * w + dj
                        nc.tensor.matmul(
                            psav[0:DH + 1, 0:TOK], v_full[:, j, h, :],
                            E16[:, dj, :], start=(j == 0), stop=(j == GT - 1))
                rn = sp.tile([65, TOK], f16, tag="rn")
                with nc.allow_low_precision(reason="softmax denom fp16 ok at 2e-2 tol"):
                    nc.vector.reciprocal(rn[64:65, :], psav[64:65, 0:TOK])
                rn_d = dmp.tile([1, TOK], f16, tag="rn_d")
                nc.sync.dma_start(rn_d[:], rn[64:65, :])
                rnb = sp.tile([DH, TOK], f16, tag="rnb")
                nc.sync.dma_start(rnb[:], rn_d[0, None, :].to_broadcast((DH, TOK)))
                dst = oT[0:DH, ft, :] if h % 2 == 0 else oTmp[:, ft % 4, :]
                nc.vector.tensor_mul(dst, psav[0:DH, 0:TOK], rnb[:])
                if h % 8 == 7:
                    nc.sync.dma_start(
                        oT[DH:P, 4 * (h // 8):4 * (h // 8) + 4, :], oTmp[:])
                    # Wout half-pass: heads (h-7)..h ready -> overlap rest
                    oh = h // 8
                    for db in range(KT):
                        ps = ps_a.tile([P, 512], f32, tag="ps_a")
                        for o4 in range(4):
                            ot = 4 * oh + o4
                            nc.tensor.matmul(
                                ps[:, 0:TOK], wout_t[:, ot, db * P:(db + 1) * P],
                                oT[:, ot, :], start=(o4 == 0), stop=(o4 == 3))
                        nc.vector.tensor_add(xT[:, db, :], xT[:, db, :],
                                             ps[:, 0:TOK])

            # ---- FFN ----
            h2 = hp.tile([P, KT, TOK], f16, tag="h16")
            layernorm_T(h2)
            for ffh in range(2):
                g16 = gp.tile([P, KT, TOK], f16, tag="g16")
                for fo in range(KT):
                    ffb = ffh * KT + fo
                    ps = ps_a.tile([P, 512], f32, tag="ps_a")
                    for kt in range(KT):
                        nc.tensor.matmul(
                            ps[:, 0:TOK], w1_t[:, kt, ffb * P:(ffb + 1) * P],
                            h2[:, kt, :], start=(kt == 0), stop=(kt == KT - 1))
                    nc.scalar.activation(
                        g16[:, fo, :], ps[:, 0:TOK],
                        AF.Sigmoid if SIM_GELU_SUBST else AF.Gelu,
                        bias=b1_t[:, l, ffb:ffb + 1])
                for db in range(KT):
                    ps = ps_a.tile([P, 512], f32, tag="ps_a")
                    for fo in range(KT):
                        nc.tensor.matmul(
                            ps[:, 0:TOK],
                            w2_t[:, ffh * KT + fo, db * P:(db + 1) * P],
                            g16[:, fo, :], start=(fo == 0), stop=(fo == KT - 1))
                    nc.vector.tensor_add(xT[:, db, :], xT[:, db, :], ps[:, 0:TOK])

        # ---- final layernorm (feature-major) + transpose + output ----
        hf = hp.tile([P, KT, TOK], f16, tag="h16")
        layernorm_T(hf, final=True)
        for s in range(2):
            o32 = gp.tile([P, KT, P], f32, tag="g16")
            for kt in range(KT):
                ps = ps_a.tile([P, 512], f32, tag="ps_a")
                nc.tensor.matmul(ps[:, 0:P], hf[:, kt, s * P:(s + 1) * P],
                                 ident[:], start=True, stop=True)
                nc.vector.tensor_copy(o32[:, kt, :], ps[:, 0:P])
            nc.sync.dma_start(
                dp["out"][s * P:(s + 1) * P, :].rearrange(
                    "t (kt p) -> t kt p", p=P), o32[:])


# ======================= host side =======================

def _prep_inputs(input_ids, token_emb, pos_emb, ln1_s, ln1_b, Wqkv, Wout,
                 ln2_s, ln2_b, W1, W2, lnf_s, lnf_b):
    emb16 = np.asarray(token_emb, np.float16)
    pos32 = np.asarray(pos_emb, np.float32)
    ids_np = np.asarray(input_ids).astype(np.int32)
    Wqkv64 = np.asarray(Wqkv, np.float64)
    W164 = np.asarray(W1, np.float64)
    Wqkv_f = Wqkv64 * np.asarray(ln1_s, np.float64)[:, :, None]
    bqkv_f = np.einsum("ld,ldn->ln", np.asarray(ln1_b, np.float64), Wqkv64)
    W1_f = W164 * np.asarray(ln2_s, np.float64)[:, :, None]
    b1_f = np.einsum("ld,ldn->ln", np.asarray(ln2_b, np.float64), W164)

    wqkv16 = Wqkv_f.astype(np.float16)
    wout16 = np.asarray(Wout, np.float16)
    w116 = W1_f.astype(np.float16)
    w216 = np.asarray(W2, np.float16)
    bqk = bqkv_f[:, 0:2 * D].reshape(L, 16, P).astype(np.float32)
    bv = bqkv_f[:, 2 * D:].astype(np.float16)
    b1 = b1_f.reshape(L, 16, P).astype(np.float32)
    lnf = np.stack([np.asarray(lnf_s, np.float32),
                    np.asarray(lnf_b, np.float32)]).reshape(2, KT, P)
    ident_np = np.eye(P, dtype=np.float16)
    tri = np.triu(np.ones((P, P), np.float32))  # [k, q]: 1 where k <= q

    in_maps = []
    for core in range(NCORES):
        g, r = divmod(core, NG)
        tok0 = r * TOK
        masks = np.zeros((P, GT, 2, P), np.float32)
        for j in range(GT):
            for s in range(2):
                gq = 2 * r + s
                if j < gq:
                    masks[:, j, s, :] = 1.0
                elif j == gq:
                    masks[:, j, s, :] = tri
        in_maps.append(dict(
            emb=emb16,
            ids=ids_np[g, tok0:tok0 + TOK][:, None],
            pos=pos32[tok0:tok0 + TOK].astype(np.float16),
            wqkv=wqkv16, wout=wout16, w1=w116, w2=w216,
            bqk=bqk, bv=bv, b1=b1, lnf=lnf,
            masks=masks.astype(np.float16), ident=ident_np,
        ))
    return in_maps


# ---------- compile-once / run-many PJRT runner (vendored) ----------

class SpmdRunner:
    def __init__(self, nc, n_cores=8):
        import jax
        from jax.sharding import Mesh, PartitionSpec
        from jax.experimental.shard_map import shard_map
        from concourse.bass2jax import (
            _bass_exec_p, install_neuronx_cc_hook, partition_id_tensor)
        self.jax = jax
        self.PartitionSpec = PartitionSpec
        install_neuronx_cc_hook()
        if not nc.is_finalized():
            nc.finalize()
        self.n_cores = n_cores
        partition_name = (
            nc.partition_id_tensor.name if nc.partition_id_tensor else None)
        in_names, out_names, out_avals, zero_outs = [], [], [], []
        for alloc in nc.m.functions[0].allocations:
            if not isinstance(alloc, mybir.MemoryLocationSet):
                continue
            name = alloc.memorylocations[0].name
            if alloc.kind == "ExternalInput":
                if name != partition_name:
                    in_names.append(name)
            elif alloc.kind == "ExternalOutput":
                out_names.append(name)
                shape = tuple(alloc.tensor_shape)
                dtype = mybir.dt.np(alloc.dtype)
                out_avals.append(jax.core.ShapedArray(shape, dtype))
                zero_outs.append(np.zeros(shape, dtype))
        self.in_names, self.out_names = in_names, out_names
        self.out_avals, self.zero_outs = out_avals, zero_outs
        n_params, n_outs = len(in_names), len(out_avals)
        self.n_params = n_params
        all_in = in_names + out_names + (
            [partition_name] if partition_name else [])
        donate = tuple(range(n_params, n_params + n_outs))

        def _b(*args):
            ops = list(args)
            if partition_name:
                ops.append(partition_id_tensor())
            return tuple(_bass_exec_p.bind(
                *ops, out_avals=tuple(out_avals), in_names=tuple(all_in),
                out_names=tuple(out_names), lowering_input_output_aliases=(),
                sim_require_finite=True, sim_require_nnan=True, nc=nc))

        devices = jax.devices()[:n_cores]
        self.mesh = Mesh(np.asarray(devices), ("core",))
        specs = (PartitionSpec("core"),)
        self.sharded = jax.jit(
            shard_map(_b, mesh=self.mesh,
                      in_specs=specs * (n_params + n_outs),
                      out_specs=specs * len(out_names), check_rep=False),
            donate_argnums=donate, keep_unused=True)
        self._dev_inputs = None

    def _zeros(self):
        return [np.zeros((self.n_cores * z.shape[0], *z.shape[1:]), z.dtype)
                for z in self.zero_outs]

    def stage_inputs(self, in_maps):
        jax, PS = self.jax, self.PartitionSpec
        per_core = [[np.asarray(m[n]) for n in self.in_names] for m in in_maps]
        concat = [np.concatenate([per_core[c][i] for c in range(self.n_cores)],
                                 axis=0) for i in range(self.n_params)]
        sh = jax.sharding.NamedSharding(self.mesh, PS("core"))
        self._dev_inputs = [jax.device_put(a, sh) for a in concat]
        for a in self._dev_inputs:
            a.block_until_ready()

    def run(self, in_maps=None):
        if in_maps is not None:
            self.stage_inputs(in_maps)
        outs = self.sharded(*self._dev_inputs, *self._zeros())
        out_np = [np.asarray(a) for a in outs]
        return [{n: out_np[i].reshape(self.n_cores, *self.out_avals[i].shape)[c]
                 for i, n in enumerate(self.out_names)}
                for c in range(self.n_cores)]

    def time_exec(self, iters=8, warmup=2):
        jax, PS = self.jax, self.PartitionSpec
        sh = jax.sharding.NamedSharding(self.mesh, PS("core"))
        zsets = [[jax.device_put(z, sh) for z in self._zeros()]
                 for _ in range(warmup + iters)]
        for zs in zsets:
            for z in zs:
                z.block_until_ready()
        outs = []
        for i in range(warmup):
            outs.append(self.sharded(*self._dev_inputs, *zsets[i]))
        for o in outs[-1]:
            o.block_until_ready()
        t0 = time.perf_counter()
        outs = []
        for i in range(iters):
            outs.append(self.sharded(*self._dev_inputs, *zsets[warmup + i]))
        for o in outs[-1]:
            o.block_until_ready()
        return (time.perf_counter() - t0) / iters


_RUNNER = None


def get_runner():
    global _RUNNER
    if _RUNNER is None:
        _RUNNER = SpmdRunner(build_nc(), NCORES)
    return _RUNNER


def kernel(**inputs) -> np.ndarray:
    in_maps = _prep_inputs(**{k: np.asarray(v) for k, v in inputs.items()})
    res = get_runner().run(in_maps)
    out = np.empty((B, T, D), np.float32)
    for core in range(NCORES):
        g, r = divmod(core, NG)
        out[g, r * TOK:(r + 1) * TOK] = res[core]["out"]
    return out
